# revision 1
# baseline (speedup 1.0000x reference)
"""Multi-head attention TRN2 kernel (8 NeuronCores, SPMD).

Sharding: data parallel over batch (4) x tensor parallel over head halves
(2 groups of 8 heads) = 8 shards. Each core computes, for its (batch,
head-group):
  xT = x.T                      (PE transposes)
  Q^T = (wq*0.125).T @ x.T + bq*0.125   [512, 2048]  (scale folded on host)
  K^T = wk.T @ x.T                      [512, 2048]  (bk dropped: softmax-invariant)
  V   = x @ wv + bv                     [2048, 512]
  per head: S^T = K_h @ Q_h^T  (row-tiled head pairs, K=64)
            P^T = exp(S^T)     (no max subtraction; scores are O(1) here)
            ctx'^T = [V_h | 1].T @ P^T  [65, 2048]  (row 64 = softmax sums)
            y_h = Wf_h.T @ ctx'^T[0:64] [64, 2048]  (zero-padded K=128)
Host combines: out_b = sum_h (y_h / sums_h).T + bf.

Bias bk is mathematically irrelevant: scores_ij = (q_i+bq)(k_j+bk) and the
q_i*bk + bq*bk terms are constant in j, cancelling in the row softmax.
"""

import json
import os
import sys
import types

import numpy as np

# ---------------------------------------------------------------------------
# Environment shims (walrus sync-wait limit + optional NTFF profile hook)
# ---------------------------------------------------------------------------

_patched = False


def _ensure_patches():
    global _patched
    if _patched:
        return
    import concourse.bass_utils as bass_utils
    import concourse.bass2jax as bass2jax
    import concourse.tile as tile
    from concourse.vector_clock import ScopedClock

    MAX_WAITS = 1
    MARK = "__waits_split__"

    def _split(bir_json: bytes) -> bytes:
        d = json.loads(bir_json)
        if d.get(MARK):
            return bir_json
        n_new = 0
        for fn in d.get("functions", []):
            for bb in fn.get("blocks", []):
                insts = bb.get("instructions", [])
                out = []
                for inst in insts:
                    si = inst.get("sync_info")
                    waits = (si or {}).get("on_wait") or []
                    if len(waits) > MAX_WAITS:
                        extra = waits[:-MAX_WAITS]
                        si["on_wait"] = waits[-MAX_WAITS:]
                        for k in range(0, len(extra), MAX_WAITS):
                            out.append({
                                "name": f"WSP-{n_new}",
                                "opcode": "NoOp",
                                "engine": inst["engine"],
                                "ins": [],
                                "outs": [],
                                "text_hint": "wait_split",
                                "sync_info": {
                                    "on_wait": extra[k:k + MAX_WAITS],
                                    "on_update": [],
                                },
                            })
                            n_new += 1
                    out.append(inst)
                if len(out) != len(insts):
                    bb["instructions"] = out
        d[MARK] = True
        return json.dumps(d).encode()

    orig_compile = bass_utils.compile_bir_kernel

    def patched_compile(bir_json, tmpdir, neff_name="file.neff"):
        return orig_compile(_split(bir_json), tmpdir, neff_name)

    bass_utils.compile_bir_kernel = patched_compile
    if getattr(bass2jax, "compile_bir_kernel", None) is not None:
        bass2jax.compile_bir_kernel = patched_compile

    def _drain_and_barrier(self, tick_clock, wait_clock):
        nc = self.nc
        probe = nc.sync.nop(nofuse=True, hint="drain_waits_probe")
        wait_clock.add_sem_waits(
            probe.ins, ScopedClock({None: tick_clock.global_clock})
        )
        nc.sync.drain()
        nc.all_engine_barrier()
        assert self.sems is not None
        popped = nc._tile_sem_poison_stack.pop()
        assert popped is self._sem_poison
        nc.clear_and_free_semaphores(list(self.sems.allocated().values()))
        nc.all_engine_barrier()

    tile.TileContext._drain_and_barrier = _drain_and_barrier
    _patched = True


def _ensure_profile_hook():
    """Register the ctypes NTFF hook so trace=True works under axon."""
    try:
        import antenv
    except ImportError:
        return
    if "antenv.axon_hooks" not in sys.modules:
        m = types.ModuleType("antenv.axon_hooks")
        m._hook = None
        m.set_axon_ntff_profile_hook = lambda h: setattr(m, "_hook", h)
        m.get_axon_ntff_profile_hook = lambda: m._hook
        sys.modules["antenv.axon_hooks"] = m
        antenv.axon_hooks = m
    mod = sys.modules["antenv.axon_hooks"]
    if mod.get_axon_ntff_profile_hook() is None:
        try:
            from trn_agent_boot.trn_boot import _ntff_profile_via_ctypes
            mod.set_axon_ntff_profile_hook(
                _ntff_profile_via_ctypes("/opt/axon/libaxon_pjrt.so")
            )
        except Exception:
            pass


# ---------------------------------------------------------------------------
# Problem constants (hardcoded per contract)
# ---------------------------------------------------------------------------

B, S, DIN = 4, 2048, 1024
H, D = 16, 64
PROJ = H * D          # 1024
NCORES = 8
PL = PROJ // 2        # 512 per-core projection (8 heads)
HL = 8                # local heads
NPAIR = 4             # local head pairs
ST = S // 128         # 16 seq tiles
KT = DIN // 128       # 8 contraction tiles
VSTRIDE = HL * 65     # 520 columns of V(+ones) per seq tile

_cache = {}


def _build_program():
    import concourse.bass as bass
    import concourse.mybir as mybir
    import concourse.tile as tile
    from concourse.masks import make_identity

    f32 = mybir.dt.float32
    f32r = mybir.dt.float32r
    f16 = mybir.dt.float16
    EXP = mybir.ActivationFunctionType.Exp

    nc = bass.Bass("TRN2", target_bir_lowering=False, debug=False)

    x_d = nc.dram_tensor("x", [S, DIN], f32, kind="ExternalInput")
    cst_d = nc.dram_tensor("cst", [128, 1152], f32r, kind="ExternalInput")
    # wq/wk host-pre-tiled to [kt, mt, 128, 128] so each block DMA is
    # row-contiguous (column-slicing [1024, 512] costs 512B strided lines)
    wq_d = nc.dram_tensor("wq", [KT, NPAIR, 128, 128], f32r, kind="ExternalInput")
    wk_d = nc.dram_tensor("wk", [KT, NPAIR, 128, 128], f32r, kind="ExternalInput")
    wv_d = nc.dram_tensor("wv", [DIN, PL], f32r, kind="ExternalInput")
    bq_d = nc.dram_tensor("bq", [PL], f32, kind="ExternalInput")
    bvb_d = nc.dram_tensor("bvb", [128, PL], f32, kind="ExternalInput")
    wf_d = nc.dram_tensor("wf", [HL, 128, D], f32r, kind="ExternalInput")
    y_d = nc.dram_tensor("y", [HL, D, S], f32, kind="ExternalOutput")
    s_d = nc.dram_tensor("s", [HL, S], f32, kind="ExternalOutput")

    with tile.TileContext(nc) as tc:
        with (
            tc.tile_pool(name="big", bufs=1) as big,
            tc.tile_pool(name="qk", bufs=2) as qkpool,
            tc.tile_pool(name="xload", bufs=4) as xload,
            tc.tile_pool(name="wblk", bufs=6) as wblk,
            tc.tile_pool(name="wvblk", bufs=4) as wvblk,
            tc.tile_pool(name="pt", bufs=6) as ptpool,
            tc.tile_pool(name="yst", bufs=2) as yst,
            tc.tile_pool(name="ps", bufs=1, space="PSUM") as ps,
            tc.tile_pool(name="ps2", bufs=2, space="PSUM") as ps2,
        ):
            # Persistent SBUF tensors
            ident = big.tile([128, 128], f32, tag="ident")
            xt = big.tile([128, KT * S], f32r, tag="xt")          # x^T
            vt = big.tile([128, ST * VSTRIDE], f16, tag="vt")     # V (+ones)
            bqt = big.tile([128, NPAIR], f32, tag="bqt")
            bvt = big.tile([128, PL], f32, tag="bvt")
            wft = big.tile([128, HL * D], f32r, tag="wft")
            cxs = []
            for i in range(4):
                t = big.tile([128, 512], f32r, tag=f"cxs{i}")     # ctx^T staging
                cxs.append(t)

            make_identity(nc, ident[:])

            # PSUM tag map (8 banks total):
            #   ps:  q0 q1 (QK accs / pre-phase rotation)  k0 k1 (ctx / FC)
            #   ps2: s x2 bufs (S^T double-buffered)
            TAGS = ["q0", "q1", "k0", "k1"]

            # ---- Phase 1: x -> x^T via PE transposes --------------------
            # (x DMAs issued first — everything else waits on them)
            for st in range(ST):
                xtile = xload.tile([128, DIN], f32, tag="xtile")
                nc.sync.dma_start(xtile[:, 0:512], x_d[st * 128:(st + 1) * 128, 0:512])
                nc.sync.dma_start(xtile[:, 512:1024], x_d[st * 128:(st + 1) * 128, 512:1024])
                for kt in range(KT):
                    tp = ps.tile([128, 128], f32, tag=TAGS[kt % 4])
                    nc.tensor.transpose(tp[:], xtile[:, kt * 128:(kt + 1) * 128], ident[:])
                    nc.vector.tensor_copy(
                        xt[:, kt * S + st * 128: kt * S + (st + 1) * 128], tp[:]
                    )

            # constants / biases (needed later than x)
            ones_view = vt[:].rearrange("p (t c) -> p t c", c=65)[:, :, 64:65]
            onesrc = xload.tile([128, 128], f32r, tag="ones")
            nc.sync.dma_start(onesrc[:], cst_d[:, 0:128])
            nc.vector.tensor_copy(
                ones_view, onesrc[:].rearrange("p (t c) -> p t c", c=1)
            )
            # ctx staging rows 64..127 stay zero (zero-padded FC contraction)
            for t in cxs:
                nc.sync.dma_start(t[:], cst_d[:, 128:640])

            nc.sync.dma_start(bqt[:], bq_d[:].rearrange("(t p) -> p t", p=128))
            nc.sync.dma_start(bvt[:], bvb_d[:])
            for h in range(HL):
                nc.sync.dma_start(wft[:, h * D:(h + 1) * D], wf_d[h, :, :])

            # ---- Phase 2: QK projections for one pair -------------------
            def emit_qk(p):
                """Q^T and K^T tiles [128, S] for head pair p.

                Uses only the two q0/q1 banks so it can overlap the
                attention phase (which owns sA/sB/k0/k1)."""
                qt_p = qkpool.tile([128, S], f32r, tag="qt")
                kt_p = qkpool.tile([128, S], f32r, tag="ktr")
                for w_d_, dst, is_q in ((wq_d, qt_p, True), (wk_d, kt_p, False)):
                    for ntg in range(2):
                        accs = []
                        for i in range(2):
                            acc = ps.tile([128, 512], f32, tag=f"q{i}")
                            accs.append(acc)
                        for kt in range(KT):
                            wb = wblk.tile([128, 128], f32r, tag="wb")
                            nc.sync.dma_start(wb[:], w_d_[kt, p, :, :])
                            for i in range(2):
                                nt = ntg * 2 + i
                                nc.tensor.matmul(
                                    accs[i][:], wb[:],
                                    xt[:, kt * S + nt * 512: kt * S + (nt + 1) * 512],
                                    start=(kt == 0), stop=(kt == KT - 1),
                                )
                        for i in range(2):
                            nt = ntg * 2 + i
                            if is_q:
                                nc.vector.tensor_scalar_add(
                                    dst[:, nt * 512:(nt + 1) * 512], accs[i][:],
                                    bqt[:, p:p + 1],
                                )
                            else:
                                nc.vector.tensor_copy(
                                    dst[:, nt * 512:(nt + 1) * 512], accs[i][:]
                                )
                return qt_p, kt_p

            # ---- Phase 2b: V = x @ wv + bv ------------------------------
            def emit_v2():
                # only k0/k1 banks, so QK projections (q0/q1) overlap freely
                for stg in range(8):
                    accs = []
                    for sti in range(2):
                        a = ps.tile([128, PL], f32, tag=f"k{sti}")
                        accs.append(a)
                    for kt in range(KT):
                        wb = wvblk.tile([128, PL], f32r, tag="wv")
                        nc.sync.dma_start(wb[:], wv_d[kt * 128:(kt + 1) * 128, :])
                        for sti in range(2):
                            st = stg * 2 + sti
                            nc.tensor.matmul(
                                accs[sti][:],
                                xt[:, kt * S + st * 128: kt * S + (st + 1) * 128],
                                wb[:],
                                start=(kt == 0), stop=(kt == KT - 1),
                            )
                    for sti in range(2):
                        st = stg * 2 + sti
                        for h in range(HL):
                            off = st * VSTRIDE + h * 65
                            nc.vector.tensor_tensor(
                                vt[:, off:off + 64],
                                accs[sti][:, h * 64:(h + 1) * 64],
                                bvt[:, h * 64:(h + 1) * 64],
                                op=mybir.AluOpType.add,
                            )

            # ---- Phase 3: attention for one pair, one 512-q chunk -------
            unit_no = [0]

            def emit_attention_unit(p, qc, qt_p, kt_p):
                q0 = qc * 512
                ctx = []
                for h in range(2):
                    t = ps.tile([65, 512], f32, tag=f"k{h}")
                    ctx.append(t)
                for kt_i in range(ST):   # 16 kseq tiles
                    # both heads' S^T chunks side by side in one 2-bank tile
                    s_ps = ps2.tile([128, 1024], f32, tag="s")
                    for h in range(2):
                        r0 = h * 64
                        nc.tensor.matmul(
                            s_ps[:, h * 512:(h + 1) * 512],
                            kt_p[r0:r0 + 64, kt_i * 128:(kt_i + 1) * 128],
                            qt_p[r0:r0 + 64, q0:q0 + 512],
                            start=True, stop=True,
                            tile_position=(r0, 0),
                        )
                    pt_t = ptpool.tile([128, 1024], f16, tag="pt")
                    nc.scalar.activation(pt_t[:], s_ps[:], EXP)
                    for h in range(2):
                        gh = p * 2 + h
                        off = kt_i * VSTRIDE + gh * 65
                        nc.tensor.matmul(
                            ctx[h][:], vt[:, off:off + 65],
                            pt_t[:, h * 512:(h + 1) * 512],
                            start=(kt_i == 0), stop=(kt_i == ST - 1),
                        )
                # FC + outputs for this chunk
                for h in range(2):
                    gh = p * 2 + h
                    cx = cxs[(unit_no[0] % 2) * 2 + h]
                    # rows 0:64 = ctx^T, row 64 = softmax sums
                    # (wf row 64 is zero, so it drops out of the FC)
                    nc.vector.tensor_copy(cx[0:65, :], ctx[h][0:65, :])
                    nc.sync.dma_start(
                        s_d[gh, q0:q0 + 512], cx[64:65, :].bitcast(f32)
                    )
                    yp = ps.tile([64, 512], f32, tag=f"k{h}")
                    nc.tensor.matmul(
                        yp[:], wft[:, gh * 64:(gh + 1) * 64], cx[:],
                        start=True, stop=True,
                    )
                    yo = yst.tile([64, 512], f32, tag="yo")
                    nc.vector.tensor_copy(yo[:], yp[:])
                    nc.sync.dma_start(y_d[gh, :, q0:q0 + 512], yo[:])
                unit_no[0] += 1

            qk_next = emit_qk(0)
            emit_v2()
            for p in range(NPAIR):
                qk_cur = qk_next
                for qc in range(4):
                    emit_attention_unit(p, qc, *qk_cur)
                    if qc == 0 and p + 1 < NPAIR:
                        qk_next = emit_qk(p + 1)

    return nc


def _prepare_in_maps(x, Wq, bq, Wk, bk, Wv, bv, Wf, bf):
    in_maps = []
    for core in range(NCORES):
        b, g = core // 2, core % 2
        sl = slice(g * PL, (g + 1) * PL)
        wf_s = np.zeros((HL, 128, D), dtype=np.float32)
        for h in range(HL):
            wf_s[h, :D, :] = Wf[g * PL + h * D: g * PL + (h + 1) * D, :]
        cst = np.zeros((128, 1152), dtype=np.float32)
        cst[:, 0:128] = 1.0
        def _tile_w(w):  # [1024, 512] -> [kt, mt, 128, 128]
            return np.ascontiguousarray(
                w.reshape(KT, 128, NPAIR, 128).transpose(0, 2, 1, 3)
            )
        in_maps.append({
            "x": np.ascontiguousarray(x[b]),
            "cst": cst,
            "wq": _tile_w(Wq[:, sl] * 0.125),
            "wk": _tile_w(Wk[:, sl]),
            "wv": np.ascontiguousarray(Wv[:, sl]),
            "bq": np.ascontiguousarray(bq[sl] * 0.125),
            "bvb": np.broadcast_to(bv[sl], (128, PL)).copy(),
            "wf": wf_s,
        })
    return in_maps


def kernel(**inputs):
    _ensure_patches()
    _ensure_profile_hook()
    from concourse.bass_utils import run_bass_kernel_spmd

    if "nc" not in _cache:
        _cache["nc"] = _build_program()
    nc = _cache["nc"]

    inp = {k: np.asarray(v, dtype=np.float32) for k, v in inputs.items()}
    in_maps = _prepare_in_maps(**inp)

    trace = bool(os.environ.get("MHA_TRACE"))
    res = run_bass_kernel_spmd(nc, in_maps, list(range(NCORES)), trace=trace)
    _cache["last_results"] = res

    bf = inp["bf"]
    out = np.empty((B, S, D), dtype=np.float32)
    for b in range(B):
        acc = np.zeros((D, S), dtype=np.float64)
        for core in (2 * b, 2 * b + 1):
            yc = res.results[core]["y"]      # [8, 64, 2048]
            sc = res.results[core]["s"]      # [8, 2048]
            acc += (yc.astype(np.float64) / sc[:, None, :]).sum(axis=0)
        out[b] = acc.T + bf
    return out

